# revision 11
# baseline (speedup 1.0000x reference)
"""Trainium2 Bass kernel for nn_Cross_head (sparse_attention patch-correction).

Math (non-overlapping unfold/fold are inverse permutations):
    y   = W @ x + b                   (1x1x1 conv over channels)
    out = leaky_relu(y * (y + 1 + A), 0.2),  A = att / (count_nonzero(att) + 1e-5)

Factorization used on device (q read once from PSUM by the scalar engine):
    q   = psum + (b+1)        # scalar engine, PSUM -> SBUF fp16
    A   = att * r             # r = 1/nz broadcast per patch column
    t   = A + q               # = y + 1 + A
    p   = q - 1               # = y
    pre = t * p
    out = prelu(pre, 0.2)

Sharding: spatial across the 576 patch columns (72 per core), no cross-core
communication.  All I/O is fp16 (host casts/packs), every DMA is contiguous
per channel (5832B descriptors), pure HWDGE on the sync queue.

Per-subtile free layout is (pq=81, p2-major inside, iw=36) so that every
element-wise operand is innermost-packed fp16 in SBUF: TT ops run in DVE 2x
mode, TS ops in 4x mode.  The 1/nz broadcast operand is packed on its
innermost (iw) dim with stride-0 only on the outer pq dim, which keeps 2x.
The nz count-reduce reads st=(att!=0) with a strided view (no fast mode for
reduce anyway).
"""

import os
import sys

import numpy as np

sys.path.insert(0, "/opt/trn_rl_repo")

# ---- geometry (hardcoded for this problem) ----
C = 128          # channels (in == out)
D = 36           # depth
HWFULL = 5184    # H*W = 72*72
PS = 9           # patch size
PQ = PS * PS     # 81 kernel positions
NDP = 4          # D // PS
NWP = 576        # HWFULL // PS  (patch columns)
NCORES = 8
IWG = NWP // NCORES   # 72 patch columns per core
NSUB = 2              # split each iD block into halves along iW
IWT = IWG // NSUB     # 36 patch columns per subtile
FT = IWT * PQ         # 2916 elements per subtile per partition
NT = NDP * NSUB       # 8 subtiles
MMN = 486             # matmul free dim (2916 / 6)
NMM = 6               # matmuls per subtile
NGRP = 2              # psum groups per subtile
MMG = NMM // NGRP     # 3 matmuls per psum group
BANK = 512            # fp32 elements per PSUM bank

_NC_CACHE = {}
LAST_RESULT = None


def _build_nc(ne_engine="vector", amul_engine="gpsimd", prelu_engine="scalar",
              p_engine="vector", nz_engine="tensor"):
    from contextlib import ExitStack

    import concourse.bacc as bacc
    import concourse.tile as tile
    from concourse import mybir

    f32 = mybir.dt.float32
    f16 = mybir.dt.float16
    AL = mybir.AluOpType
    AF = mybir.ActivationFunctionType

    nc = bacc.Bacc(
        "TRN2",
        target_bir_lowering=False,
        debug=False,
        enable_asserts=False,
        num_devices=NCORES,
    )
    x_d = nc.dram_tensor("x", [C, NT, FT], f16, kind="ExternalInput").ap()
    a_d = nc.dram_tensor("att", [C, NT, FT], f16, kind="ExternalInput").ap()
    wt_d = nc.dram_tensor("wt", [C, C], f16, kind="ExternalInput").ap()
    id_d = nc.dram_tensor("ident", [C, C], f16, kind="ExternalInput").ap()
    b_d = nc.dram_tensor("bias", [C, 2], f32, kind="ExternalInput").ap()
    o_d = nc.dram_tensor("out", [C, NT, FT], f16, kind="ExternalOutput").ap()

    with tile.TileContext(nc) as tc, ExitStack() as ctx:
        const = ctx.enter_context(tc.tile_pool(name="const", bufs=1))
        wt_sb = const.tile([C, C], f16)
        nc.sync.dma_start(wt_sb[:], wt_d[:])
        id_sb = const.tile([C, C], f16)
        nc.sync.dma_start(id_sb[:], id_d[:])
        b_sb = const.tile([C, 2], f32)
        nc.sync.dma_start(b_sb[:], b_d[:])
        b_ap = b_sb[:, 0:1]
        bp1_ap = b_sb[:, 1:2]
        alpha_sb = const.tile([C, 1], f32)
        nc.vector.memset(alpha_sb[:], 0.2)

        xp = ctx.enter_context(tc.tile_pool(name="xp", bufs=3))
        atp = ctx.enter_context(tc.tile_pool(name="atp", bufs=3))
        stp = ctx.enter_context(tc.tile_pool(name="stp", bufs=2))
        nzp = ctx.enter_context(tc.tile_pool(name="nzp", bufs=2))
        Apl = ctx.enter_context(tc.tile_pool(name="Apl", bufs=2))
        qpl = ctx.enter_context(tc.tile_pool(name="qpl", bufs=2))
        tpl = ctx.enter_context(tc.tile_pool(name="tpl", bufs=2))
        ppl = ctx.enter_context(tc.tile_pool(name="ppl", bufs=2))
        prp = ctx.enter_context(tc.tile_pool(name="prp", bufs=2))
        ovp = ctx.enter_context(tc.tile_pool(name="ovp", bufs=3))
        psp = ctx.enter_context(tc.tile_pool(name="psp", bufs=2, space="PSUM"))
        nzps = (
            ctx.enter_context(tc.tile_pool(name="nzps", bufs=2, space="PSUM"))
            if nz_engine == "tensor"
            else None
        )

        ne_eng = {"vector": nc.vector, "gpsimd": nc.gpsimd}.get(ne_engine)
        amul = {"vector": nc.vector, "gpsimd": nc.gpsimd}[amul_engine]

        def issue_loads(sub):
            xt = xp.tile([C, FT], f16, name=f"xt{sub}", tag="xt")
            nc.sync.dma_start(xt[:], x_d[:, sub, :])
            at = atp.tile([C, FT], f16, name=f"at{sub}", tag="at")
            nc.sync.dma_start(at[:], a_d[:, sub, :])
            return xt, at

        loaded = {0: issue_loads(0), 1: issue_loads(1)}

        for sub in range(NT):
            xt, at = loaded.pop(sub)
            if sub + 2 < NT:
                loaded[sub + 2] = issue_loads(sub + 2)

            # ---- nz = count_nonzero per patch column ----
            nzv = nzp.tile([C, IWT], f32, name=f"nz{sub}", tag="nz")
            if nz_engine == "tensor":
                # st = (att != 0) on DVE (4x); sum over p1 via 9 accumulated
                # identity matmuls on the tensor engine; sum over p2 with a
                # small 324-element DVE reduce out of PSUM.
                st = stp.tile([C, FT], f16, name=f"st{sub}", tag="st")
                nc.vector.tensor_scalar(st[:], at[:], 0.0, None, AL.not_equal)
                nzq = nzps.tile([C, PS * IWT], f32)  # 1 bank
                for p1 in range(PS):
                    nc.tensor.matmul(
                        nzq[:],
                        id_sb[:],
                        st[:, p1 * PS * IWT : (p1 + 1) * PS * IWT],
                        start=(p1 == 0),
                        stop=(p1 == PS - 1),
                    )
                nc.vector.tensor_reduce(
                    nzv[:],
                    nzq[:].rearrange("c (q w) -> c q w", q=PS).transpose([0, 2, 1]),
                    mybir.AxisListType.X,
                    AL.add,
                )
            elif ne_engine == "scalar":
                # |sign(att)| summed with absolute-value reduce
                st = stp.tile([C, FT], f16, name=f"st{sub}", tag="st")
                nc.scalar.activation(st[:], at[:], AF.Sign)
                nc.vector.tensor_reduce(
                    nzv[:],
                    st[:].rearrange("c (q w) -> c q w", q=PQ).transpose([0, 2, 1]),
                    mybir.AxisListType.X,
                    AL.add,
                    apply_absolute_value=True,
                )
            else:
                st = stp.tile([C, FT], f16, name=f"st{sub}", tag="st")
                ne_eng.tensor_scalar(st[:], at[:], 0.0, None, AL.not_equal)
                nc.vector.tensor_reduce(
                    nzv[:],
                    st[:].rearrange("c (q w) -> c q w", q=PQ).transpose([0, 2, 1]),
                    mybir.AxisListType.X,
                    AL.add,
                )
            # r = 1/nz in fp16 (the +1e-5 of the reference shifts r by
            # ~1.2e-7 relative — far below fp16 rounding, so it is dropped)
            rcp = nzp.tile([C, IWT], f32, name=f"rc{sub}", tag="rc")
            nc.vector.reciprocal_approx_fast(rcp[:], nzv[:])
            rh = nzp.tile([C, IWT], f16, name=f"rh{sub}", tag="rh")
            nc.vector.tensor_scalar(rh[:], rcp[:], 0.0, None, AL.add)

            # ---- A = att * r  (r broadcast over the 81 kernel positions) ----
            At = Apl.tile([C, FT], f16, name=f"A{sub}", tag="A")
            a3 = at[:].rearrange("c (q w) -> c q w", q=PQ)
            r3 = rh[:].unsqueeze(1).broadcast_to((C, PQ, IWT))
            amul.tensor_tensor(
                At[:].rearrange("c (q w) -> c q w", q=PQ), a3, r3, AL.mult
            )

            # ---- GEMM: psum = W @ x ----
            pst = []
            for g in range(NGRP):
                ps_t = psp.tile([C, MMG * BANK], f32)  # 3 banks
                pst.append(ps_t)
                for m in range(MMG):
                    ch = g * MMG + m
                    nc.tensor.matmul(
                        ps_t[:, m * BANK : m * BANK + MMN],
                        wt_sb[:],
                        xt[:, ch * MMN : (ch + 1) * MMN],
                        start=True,
                        stop=True,
                    )

            # ---- q = psum + (b+1), PSUM -> fp16 SBUF on the scalar engine --
            qt = qpl.tile([C, FT], f16, name=f"q{sub}", tag="q")
            for g in range(NGRP):
                ps_ap = (
                    pst[g][:]
                    .rearrange("c (m n) -> c m n", n=BANK)[:, :, 0:MMN]
                )
                q_ap = qt[:, g * MMG * MMN : (g + 1) * MMG * MMN].rearrange(
                    "c (m n) -> c m n", n=MMN
                )
                nc.scalar.activation(q_ap, ps_ap, AF.Identity, bias=bp1_ap)

            # ---- t = A + q ; p = q - 1 ; pre = t * p ----
            tt = tpl.tile([C, FT], f16, name=f"t{sub}", tag="t")
            nc.vector.tensor_tensor(tt[:], At[:], qt[:], AL.add)
            pt = ppl.tile([C, FT], f16, name=f"p{sub}", tag="p")
            if p_engine == "vector":
                nc.vector.tensor_scalar(pt[:], qt[:], 1.0, None, AL.subtract)
            else:
                nc.scalar.activation(pt[:], qt[:], AF.Identity, bias=-1.0)
            pre = prp.tile([C, FT], f16, name=f"pr{sub}", tag="pr")
            nc.vector.tensor_tensor(pre[:], tt[:], pt[:], AL.mult)

            # ---- out = lrelu(pre) ----
            ov = ovp.tile([C, FT], f16, name=f"ov{sub}", tag="ov")
            if prelu_engine == "scalar":
                nc.scalar.activation(ov[:], pre[:], AF.Prelu, alpha=alpha_sb[:, 0:1])
            else:
                nc.vector.scalar_tensor_tensor(
                    ov[:], pre[:], 0.2, pre[:], AL.mult, AL.max
                )

            nc.sync.dma_start(o_d[:, sub, :], ov[:])

    nc.compile()
    return nc


def _get_nc(**kw):
    key = tuple(sorted(kw.items()))
    if key not in _NC_CACHE:
        _NC_CACHE[key] = _build_nc(**kw)
    return _NC_CACHE[key]


def kernel(x, attentions, W, b, **build_kw):
    global LAST_RESULT
    from concourse.bass_utils import run_bass_kernel_spmd

    x = np.asarray(x, dtype=np.float32)
    attentions = np.asarray(attentions, dtype=np.float32)
    W = np.asarray(W, dtype=np.float32)
    b = np.asarray(b, dtype=np.float32)

    nc = _get_nc(**build_kw)

    # x: [1, C, D, HW] -> (c, iD, p1, s, h, iw, p2) -> per-core (c, iD, h, p1, p2, iw)
    xs = x.reshape(C, NDP, PS, NCORES, NSUB, IWT, PS)
    # att: [1, C, L, 81] with L=(iD, s, h, iw), 81=(p1, p2)
    as_ = attentions.reshape(C, NDP, NCORES, NSUB, IWT, PS, PS)
    wt = np.ascontiguousarray(W.T.astype(np.float16))
    ident = np.eye(C, dtype=np.float16)
    bcol = np.ascontiguousarray(np.stack([b, b + 1.0], axis=1))

    in_maps = []
    for s in range(NCORES):
        xc = xs[:, :, :, s].transpose(0, 1, 3, 2, 5, 4)  # c,iD,h,p1,p2,iw
        ac = as_[:, :, s].transpose(0, 1, 2, 4, 5, 3)    # c,iD,h,p1,p2,iw
        in_maps.append(
            {
                "x": np.ascontiguousarray(xc, dtype=np.float16).reshape(C, NT, FT),
                "att": np.ascontiguousarray(ac, dtype=np.float16).reshape(C, NT, FT),
                "wt": wt,
                "ident": ident,
                "bias": bcol,
            }
        )

    res = run_bass_kernel_spmd(
        nc,
        in_maps,
        core_ids=list(range(NCORES)),
        trace=bool(os.environ.get("BASS_TRACE")),
    )
    LAST_RESULT = res

    # out: per-core [C, NT, FT] = (c, iD, h, p1, p2, iw) -> [1, C, D, HW]
    full = np.empty((C, NDP, PS, NCORES, NSUB, IWT, PS), dtype=np.float32)
    for s in range(NCORES):
        oc = res.results[s]["out"].reshape(C, NDP, NSUB, PS, PS, IWT)
        full[:, :, :, s] = oc.transpose(0, 1, 3, 2, 5, 4).astype(np.float32)
    return full.reshape(1, C, D, HWFULL)
